# revision 17
# baseline (speedup 1.0000x reference)
"""Multi-head attention (B=4, T=S=2048, E=1024, H=16) on 8 trn2 NeuronCores.

Sharding: core c handles batch b = c // 2 and head-half hh = c % 2
(8 of 16 heads).  The host pre-transposes activations and weights to
bf16 (x.T, W.T) so the kernel needs no on-chip transposes of inputs,
and folds bv/bo into a host-side constant (softmax weights sum to 1,
so the v-bias contributes (bv @ Wo.T) to every row).

On-chip dataflow per core:
  q/k proj -> qp/kp stored as fp8e4 in DoubleRow layout [128, kt2, c4, t]
  (kt1 zeroed), scores.T = kp.T-dot-qp per head via fp8 DoubleRow matmuls
  (cost-model 0.5 cyc/row), exp on ACT from PSUM [128, 1024] 4-head tiles,
  PV in ctx-natural orientation (out [t,64] per head, M=128), softmax
  denominators via ones-column matmuls (N=1), normalize on DVE with
  per-partition reciprocal scalars, PE transpose of ctx, output
  projection to natural [T, E] f32 partials summed on host.
"""

import numpy as np
import ml_dtypes

import concourse.bass as bass
import concourse.mybir as mybir
import concourse.tile as tile
from concourse.bass_utils import run_bass_kernel_spmd
from concourse.masks import make_identity

F32 = mybir.dt.float32
BF16 = mybir.dt.bfloat16
FP8 = mybir.dt.float8e4

B, T, E = 4, 2048, 1024
H = 16   # global heads
HL = 8   # heads per core
HD = 64  # head dim
EL = HL * HD  # 512 local e-dims
N_CORES = 8
TB = 256       # t-block (ctx psum granularity: 2 tc-chunks of 128)
NTB = T // TB  # 8

_CACHED = {}


def legalize_waits(nc, cap=1):
    """Hoist semaphore waits so no instruction carries more than `cap`.

    The cayman 64B ISA instruction format has a single wait slot; this
    container's walrus rejects instructions with more attached waits.
    Tile's sem assignment freely attaches several, so we split the excess
    onto standalone InstEventSemaphore carriers on the same engine.
    Also replaces the tail RANGE_CLEAR with sem-dec updates."""
    import bass_rust

    totals = {}
    names = {}
    for f in nc.m.functions:
        for bb in f.blocks:
            for ins in bb.instructions:
                si = ins.sync_info
                if si is None:
                    continue
                for u in si.on_update or []:
                    if u.sync_type == "semaphore":
                        sign = 1 if u.update_mode in ("sem-inc", "sem-add-imm") else -1
                        totals[u.id] = totals.get(u.id, 0) + sign * u.update_value
                        names[u.id] = u.ant_name

    n = 0
    for f in nc.m.functions:
        for bb in f.blocks:
            insts = bb.instructions
            out = []
            changed = False
            for ins in insts:
                if type(ins).__name__ == "InstISA" and "RANGE_CLEAR" in str(ins):
                    import re

                    m = re.search(r"range_first=(\d+) range_last=(\d+)", str(ins))
                    first, last = int(m.group(1)), int(m.group(2))
                    for sid in range(first, last + 1):
                        tot = totals.get(sid, 0)
                        if tot == 0:
                            continue
                        ev = mybir.InstEventSemaphore(name=f"I-LC{n}", ins=[], outs=[])
                        n += 1
                        ev.engine = ins.engine
                        ev.sync_info = bass_rust.SyncInfo(
                            on_wait=[],
                            on_update=[
                                bass_rust.SyncUpdate(
                                    sync_type="semaphore",
                                    id=sid,
                                    ant_name=names.get(sid, f"sem{sid}"),
                                    update_mode="sem-sub-imm",
                                    update_value=tot,
                                    update_reg=None,
                                )
                            ],
                        )
                        out.append(ev)
                    changed = True
                    continue
                si = ins.sync_info
                ws = list(si.on_wait) if (si is not None and si.on_wait) else []
                if len(ws) > cap:
                    for w in ws[: len(ws) - cap]:
                        ev = mybir.InstEventSemaphore(name=f"I-LW{n}", ins=[], outs=[])
                        n += 1
                        ev.engine = ins.engine
                        ev.sync_info = bass_rust.SyncInfo(on_wait=[w], on_update=[])
                        out.append(ev)
                    si.on_wait = ws[len(ws) - cap :]
                    changed = True
                out.append(ins)
            if changed:
                insts[:] = out
    return n


def build_program(legalize=True, n_tb=NTB, use_dr=True):
    nc = bass.Bass()

    # Host-prepped inputs: x.T and W.T in bf16; biases f32.
    qtd = nc.declare_dram_parameter("qt", [E, T], BF16, isOutput=False)
    ktd = nc.declare_dram_parameter("kt", [E, T], BF16, isOutput=False)
    vtd = nc.declare_dram_parameter("vt", [E, T], BF16, isOutput=False)
    wqtd = nc.declare_dram_parameter("wqt", [E, EL], BF16, isOutput=False)
    wktd = nc.declare_dram_parameter("wkt", [E, EL], BF16, isOutput=False)
    wvtd = nc.declare_dram_parameter("wvt", [E, EL], BF16, isOutput=False)
    wotd = nc.declare_dram_parameter("wot", [EL, E], BF16, isOutput=False)
    bqd = nc.declare_dram_parameter("bq", [EL], F32, isOutput=False)
    bkd = nc.declare_dram_parameter("bk", [EL], F32, isOutput=False)
    outd = nc.declare_dram_parameter("out", [T, E], F32, isOutput=True)

    with tile.TileContext(nc, pool_alloc_mode="queue") as tc:
        with (
            tc.tile_pool(name="singles", bufs=1) as singles,
            tc.tile_pool(name="xt", bufs=2) as xtp,
            tc.tile_pool(name="pt", bufs=8) as ptp,
            tc.tile_pool(name="ctxn", bufs=4) as ctxnp,
            tc.tile_pool(name="ctxT", bufs=2) as ctxTp,
            tc.tile_pool(name="osb", bufs=3) as osbp,
            tc.tile_pool(name="rec", bufs=2) as recp,
            tc.tile_pool(name="sc_ps", bufs=2, space="PSUM") as sc_ps,
            tc.tile_pool(name="ctx_ps", bufs=2, space="PSUM") as ctx_ps,
            tc.tile_pool(name="den_ps", bufs=1, space="PSUM") as den_ps,
            tc.tile_pool(name="misc_ps", bufs=1, space="PSUM") as misc_ps,
        ):
            # ---------------- prologue: weights / biases / consts ----------
            ident = singles.tile([128, 128], BF16)
            make_identity(nc, ident)
            ones = singles.tile([128, 1], BF16)
            nc.vector.memset(ones, 1.0)

            # W.T natural loads: wT[p, a, e'] = WT[a*128 + p, e']
            wqT = singles.tile([128, 8, EL], BF16)
            wkT = singles.tile([128, 8, EL], BF16)
            wvT = singles.tile([128, 8, EL], BF16)
            woT = singles.tile([128, 4, E], BF16)
            bq_sb = singles.tile([128, 4], F32)
            bk_sb = singles.tile([128, 4], F32)

            def load_w_chunked(dst, wd):
                # per-e-chunk DMAs so proj matmul e can start after chunk e
                for e in range(8):
                    nc.sync.dma_start(
                        out=dst[:, e, :], in_=wd[e * 128 : (e + 1) * 128, :]
                    )

            nc.sync.dma_start(out=bk_sb, in_=bkd.rearrange("(c p) -> p c", p=128))
            nc.sync.dma_start(out=bq_sb, in_=bqd.rearrange("(c p) -> p c", p=128))

            # qp/kp fp8 DoubleRow tiles, one per 512-t block:
            # [128, kt, c, t]: partition band (h%2)*64 holds head h of chunk
            # c=h//2; kt0 = the 64 head dims, kt1 stays zero.
            qp8 = [singles.tile([128, 2, 4, 512], FP8, name=f"qp8_{i}") for i in range(4)]
            kp8 = [singles.tile([128, 2, 4, 512], FP8, name=f"kp8_{i}") for i in range(4)]
            for tl in qp8 + kp8:
                nc.gpsimd.memset(tl[:, 1, :, :], 0.0)

            # vp[s-chunk]: [128 s, 512 e'] bf16
            vp = [singles.tile([128, EL], BF16, name=f"vp_{i}") for i in range(16)]

            def load_xt(xd, blk, tag, chunked=False):
                xt = xtp.tile([128, 8, 512], BF16, tag=tag, name=f"xt_{tag}{blk}")
                sl = xd[:, blk * 512 : (blk + 1) * 512]
                if chunked:
                    for e in range(8):
                        nc.sync.dma_start(
                            out=xt[:, e, :], in_=sl[e * 128 : (e + 1) * 128, :]
                        )
                else:
                    nc.sync.dma_start(
                        out=xt, in_=sl.rearrange("(a p) t -> p a t", p=128)
                    )
                return xt

            kT = {}
            qT = {}
            kT[0] = load_xt(ktd, 0, "kt", chunked=True)
            load_w_chunked(wkT, wktd)
            qT[0] = load_xt(qtd, 0, "qt", chunked=True)
            load_w_chunked(wqT, wqtd)

            def proj_fill_qk(xt, wT, b_sb, dst8, cp, tcols, pool, tag):
                """One 128-e'-chunk x 512-t psum fill + fp8 drain."""
                ps = pool.tile([128, 512], F32, tag=tag)
                for e in range(8):
                    nc.tensor.matmul(
                        ps,
                        lhsT=wT[:, e, cp * 128 : (cp + 1) * 128],
                        rhs=xt[:, e, :],
                        start=(e == 0),
                        stop=(e == 7),
                    )
                nc.vector.tensor_scalar_add(
                    out=dst8[:, 0, cp, :], in0=ps, scalar1=b_sb[:, cp : cp + 1]
                )

            def proj_fill_v(xt, sc):
                """vp[sc] = v-rows sc*128..+128 @ WvT, into misc psum."""
                ps = misc_ps.tile([128, 512], F32, tag="misc")
                for e in range(8):
                    nc.tensor.matmul(
                        ps,
                        lhsT=xt[:, e, (sc % 4) * 128 : (sc % 4 + 1) * 128],
                        rhs=wvT[:, e, :],
                        start=(e == 0),
                        stop=(e == 7),
                    )
                nc.vector.tensor_copy(out=vp[sc], in_=ps)

            # k-proj blk0 + q-proj blk0 up front (scores tb0 need them)
            for cp in range(4):
                proj_fill_qk(kT[0], wkT, bk_sb, kp8[0], cp, 512, ctx_ps, "ctx")
            kT[1] = load_xt(ktd, 1, "kt")
            for cp in range(4):
                proj_fill_qk(qT[0], wqT, bq_sb, qp8[0], cp, 512, ctx_ps, "ctx")

            # remaining loads kick off now; v/wv/wo later consumers
            nc.sync.dma_start(
                out=wvT, in_=wvtd[:, :].rearrange("(a p) e -> p a e", p=128)
            )
            nc.sync.dma_start(
                out=woT, in_=wotd[:, :].rearrange("(a p) e -> p a e", p=128)
            )

            # ---------------- attention + interleaved fillers --------------
            vT = {}

            def dummy_sep(st, opener=False):
                """Tiny bf16 matmul: separates DR groups at different PE row
                tile positions (consecutive DR matmuls with different row
                offsets wedge the device).  Writes an unread den-bank col.
                The per-tb opener (M=128, start=True) opens the den bank's
                zero region — den accumulation then relies on first-touch
                overwrite; later dummies are M=1 accumulates onto col 500."""
                if opener:
                    nc.tensor.matmul(
                        st["den"][:, 500:501],
                        lhsT=ident,
                        rhs=ones,
                        start=True,
                        stop=True,
                        skip_group_check=True,
                    )
                else:
                    nc.tensor.matmul(
                        st["den"][0:1, 500:501],
                        lhsT=ones,
                        rhs=ones,
                        start=False,
                        stop=True,
                        skip_group_check=True,
                    )

            def scores_grp(st, tb, s, g):
                """4 same-band DR score matmuls + exp -> pt tile.
                Band g holds heads 2j+g (j=0..3) at pt cols j*256."""
                dummy_sep(st)
                sc = sc_ps.tile([128, 1024], F32, tag="sc")
                psl = slice(g * 64, g * 64 + 64)
                blk, off = divmod(tb * TB, 512)
                for j in range(4):
                    c = (2 * j + g) // 2  # == j
                    if use_dr:
                        nc.tensor.matmul(
                            sc[:, j * 256 : (j + 1) * 256],
                            lhsT=kp8[s // 4][psl, :, c, (s % 4) * 128 : (s % 4) * 128 + 128],
                            rhs=qp8[blk][psl, :, c, off : off + 256],
                            start=(j % 2 == 0),
                            stop=True,
                            perf_mode=mybir.MatmulPerfMode.DoubleRow,
                            skip_group_check=True,
                        )
                    else:
                        nc.tensor.matmul(
                            sc[:, j * 256 : (j + 1) * 256],
                            lhsT=kp8[s // 4][psl, 0, c, (s % 4) * 128 : (s % 4) * 128 + 128],
                            rhs=qp8[blk][psl, 0, c, off : off + 256],
                            start=(j % 2 == 0),
                            stop=True,
                            skip_group_check=True,
                        )
                pt = ptp.tile([128, 1024], BF16, tag="pt")
                nc.scalar.activation(
                    out=pt, in_=sc, func=mybir.ActivationFunctionType.Exp,
                    scale=0.125,
                )
                return pt

            def pv_half(tb_state, s, g):
                """PV + denom matmuls for band-g heads of (tb, s)."""
                ctx_t, den_t = tb_state["ctx"], tb_state["den"]
                pt = tb_state["pt"][s][g]
                for tc in range(2):
                    for j in range(4):
                        h = 2 * j + g
                        lhsT = pt[:, j * 256 + tc * 128 : j * 256 + tc * 128 + 128]
                        nc.tensor.matmul(
                            ctx_t[tc][:, h * 64 : h * 64 + 64],
                            lhsT=lhsT,
                            rhs=vp[s][:, h * 64 : h * 64 + 64],
                            start=(s == 0 and g == 0 and j == 0),
                            stop=(s == 15),
                            skip_group_check=True,
                        )
                        nc.tensor.matmul(
                            den_t[:, tc * 8 + h : tc * 8 + h + 1],
                            lhsT=lhsT,
                            rhs=ones,
                            start=False,
                            stop=(s == 15),
                            skip_group_check=True,
                        )

            def normalize(tb_state):
                """ctx/den psum -> ctxn bf16 tiles (per-partition recip mult)."""
                rec = recp.tile([128, 16], F32, tag="rec")
                nc.vector.reciprocal(out=rec, in_=tb_state["den"][:, 0:16])
                ctxn = []
                for tc in range(2):
                    cn = ctxnp.tile([128, EL], BF16, tag="ctxn")
                    for h in range(HL):
                        nc.vector.tensor_scalar_mul(
                            out=cn[:, h * 64 : h * 64 + 64],
                            in0=tb_state["ctx"][tc][:, h * 64 : h * 64 + 64],
                            scalar1=rec[:, tc * 8 + h : tc * 8 + h + 1],
                        )
                    ctxn.append(cn)
                tb_state["ctxn"] = ctxn

            def ctxT_fill(tb_state, half):
                """PE-transpose ctxn into ctxT[:, 2 chunks, 256]."""
                if "ctxT" not in tb_state:
                    tb_state["ctxT"] = ctxTp.tile(
                        [128, 4, 256], BF16, tag="ctxT", name=f"ctxT_{tb_state['tb']}"
                    )
                tr = misc_ps.tile([128, 512], BF16, tag="misc")
                for i in range(2):
                    cp = half * 2 + i
                    for tc in range(2):
                        nc.tensor.transpose(
                            tr[:, i * 256 + tc * 128 : i * 256 + tc * 128 + 128],
                            tb_state["ctxn"][tc][:, cp * 128 : (cp + 1) * 128],
                            ident,
                        )
                nc.vector.tensor_copy(
                    out=tb_state["ctxT"][:, half * 2 : half * 2 + 2, :], in_=tr
                )

            def outproj_piece(tb_state, tcc, oh):
                """out[t-chunk, o-half]: 4 matmuls + drain (+DMA when done)."""
                tb = tb_state["tb"]
                ps = misc_ps.tile([128, 512], F32, tag="misc")
                for cp in range(4):
                    nc.tensor.matmul(
                        ps,
                        lhsT=tb_state["ctxT"][:, cp, tcc * 128 : (tcc + 1) * 128],
                        rhs=woT[:, cp, oh * 512 : (oh + 1) * 512],
                        start=(cp == 0),
                        stop=(cp == 3),
                    )
                if oh == 0:
                    tb_state["osb"] = osbp.tile(
                        [128, E], F32, tag="osb", name=f"osb_{tb}_{tcc}"
                    )
                nc.vector.tensor_copy(
                    out=tb_state["osb"][:, oh * 512 : (oh + 1) * 512], in_=ps
                )
                if oh == 1:
                    r0 = tb * TB + tcc * 128
                    nc.sync.dma_start(
                        out=outd[r0 : r0 + 128, :], in_=tb_state["osb"]
                    )

            prev = None  # tb_state of tb-1 (fillers pending)
            for tb in range(n_tb):
                st = {
                    "tb": tb,
                    "ctx": [
                        ctx_ps.tile([128, 512], F32, tag="ctx", name=f"ctx_{tb}_{i}")
                        for i in range(2)
                    ],
                    "den": den_ps.tile([128, 512], F32, tag="den", name=f"den_{tb}"),
                    "pt": {},
                }
                dummy_sep(st, opener=True)
                for s in range(16):
                    ptA = scores_grp(st, tb, s, 0)
                    # --- work block A ---
                    if tb == 0:
                        if s == 1:
                            kT[2] = load_xt(ktd, 2, "kt")
                        if s in (2, 3, 6, 7):  # k blk1 at s2-3, blk2 at s6-7
                            blk = 1 if s < 4 else 2
                            for cp in (0, 2) if s % 2 == 0 else (1, 3):
                                proj_fill_qk(
                                    kT[blk], wkT, bk_sb, kp8[blk], cp, 512,
                                    misc_ps, "misc",
                                )
                        if s in (10, 11):  # k blk3
                            for cp in (0, 2) if s % 2 == 0 else (1, 3):
                                proj_fill_qk(
                                    kT[3], wkT, bk_sb, kp8[3], cp, 512,
                                    misc_ps, "misc",
                                )
                        if s % 4 == 0:
                            vT[s // 4] = load_xt(vtd, s // 4, "vt")
                        if s >= 1:  # v-proj lags one s-chunk
                            proj_fill_v(vT[(s - 1) // 4], s - 1)
                        if s >= 3:
                            pv_half(st, s - 3, 0)
                    else:
                        if s >= 1:
                            pv_half(st, s - 1, 0)
                        if prev is not None:
                            if s == 0:
                                ctxT_fill(prev, 0)
                            elif s == 1:
                                ctxT_fill(prev, 1)
                            elif s in (2, 3, 4, 5):
                                tcc, oh = divmod(s - 2, 2)
                                outproj_piece(prev, tcc, oh)
                    ptB = scores_grp(st, tb, s, 1)
                    st["pt"][s] = (ptA, ptB)
                    # --- work block B ---
                    if tb == 0:
                        if s == 5:
                            kT[3] = load_xt(ktd, 3, "kt")
                        if s >= 3:
                            pv_half(st, s - 3, 1)
                    else:
                        if s >= 1:
                            pv_half(st, s - 1, 1)
                        if tb in (1, 3, 5) and s == 6:
                            blk = (tb + 1) // 2
                            qT[blk] = load_xt(qtd, blk, "qt")
                        if tb in (1, 3, 5) and s in (8, 10, 12, 14):
                            blk = (tb + 1) // 2
                            proj_fill_qk(
                                qT[blk], wqT, bq_sb, qp8[blk], (s - 8) // 2, 512,
                                misc_ps, "misc",
                            )
                if tb == 0:
                    proj_fill_v(vT[3], 15)
                    for s in (13, 14, 15):
                        pv_half(st, s, 0)
                        pv_half(st, s, 1)
                else:
                    pv_half(st, 15, 0)
                    pv_half(st, 15, 1)
                normalize(st)
                st["pt"] = {}  # release references
                prev = st

            # tail: tb7 epilogue
            ctxT_fill(prev, 0)
            ctxT_fill(prev, 1)
            for tcc in range(2):
                for oh in range(2):
                    outproj_piece(prev, tcc, oh)

    if legalize:
        legalize_waits(nc)
    return nc


def _make_in_maps(inputs):
    bf = ml_dtypes.bfloat16
    per_batch = {}
    for b in range(B):
        per_batch[b] = {
            "qt": np.ascontiguousarray(np.asarray(inputs["q"][b]).T).astype(bf),
            "kt": np.ascontiguousarray(np.asarray(inputs["k"][b]).T).astype(bf),
            "vt": np.ascontiguousarray(np.asarray(inputs["v"][b]).T).astype(bf),
        }
    per_half = {}
    for hh in range(2):
        esl = slice(hh * EL, (hh + 1) * EL)
        per_half[hh] = {
            "wqt": np.ascontiguousarray(np.asarray(inputs["Wq"])[esl].T).astype(bf),
            "wkt": np.ascontiguousarray(np.asarray(inputs["Wk"])[esl].T).astype(bf),
            "wvt": np.ascontiguousarray(np.asarray(inputs["Wv"])[esl].T).astype(bf),
            "wot": np.ascontiguousarray(np.asarray(inputs["Wo"])[:, esl].T).astype(bf),
            "bq": np.ascontiguousarray(np.asarray(inputs["bq"])[esl], dtype=np.float32),
            "bk": np.ascontiguousarray(np.asarray(inputs["bk"])[esl], dtype=np.float32),
        }
    in_maps = []
    for c in range(N_CORES):
        b, hh = c // 2, c % 2
        in_maps.append({**per_batch[b], **per_half[hh]})
    return in_maps


def _gather(results, inputs):
    const = (
        np.asarray(inputs["bv"], dtype=np.float32)
        @ np.asarray(inputs["Wo"], dtype=np.float32).T
        + np.asarray(inputs["bo"], dtype=np.float32)
    )
    out = np.empty((B, T, E), dtype=np.float32)
    for b in range(B):
        out[b] = results[2 * b]["out"] + results[2 * b + 1]["out"] + const[None, :]
    return out


def run(inputs, **spmd_kwargs):
    if "nc" not in _CACHED:
        _CACHED["nc"] = build_program()
    nc = _CACHED["nc"]
    in_maps = _make_in_maps(inputs)
    res = run_bass_kernel_spmd(
        nc, in_maps, core_ids=list(range(N_CORES)), **spmd_kwargs
    )
    out = _gather(res.results, inputs)
    return out, res


def kernel(**inputs) -> np.ndarray:
    out, _ = run(inputs)
    return out


# revision 18
# speedup vs baseline: 1.0178x; 1.0178x over previous
"""Multi-head attention (B=4, T=S=2048, E=1024, H=16) on 8 trn2 NeuronCores.

Sharding: core c handles batch b = c // 2 and head-half hh = c % 2
(8 of 16 heads).  The host pre-transposes activations and weights to
bf16 (x.T, W.T) so the kernel needs no on-chip transposes of inputs,
and folds bv/bo into a host-side constant (softmax weights sum to 1,
so the v-bias contributes (bv @ Wo.T) to every row).

On-chip dataflow per core:
  q/k proj -> qp/kp stored as fp8e4 in DoubleRow layout [128, kt2, c4, t]
  (kt1 zeroed), scores.T = kp.T-dot-qp per head via fp8 DoubleRow matmuls
  (cost-model 0.5 cyc/row), exp on ACT from PSUM [128, 1024] 4-head tiles,
  PV in ctx-natural orientation (out [t,64] per head, M=128), softmax
  denominators via ones-column matmuls (N=1), normalize on DVE with
  per-partition reciprocal scalars, PE transpose of ctx, output
  projection to natural [T, E] f32 partials summed on host.
"""

import numpy as np
import ml_dtypes

import concourse.bass as bass
import concourse.mybir as mybir
import concourse.tile as tile
from concourse.bass_utils import run_bass_kernel_spmd
from concourse.masks import make_identity

F32 = mybir.dt.float32
BF16 = mybir.dt.bfloat16
FP8 = mybir.dt.float8e4

B, T, E = 4, 2048, 1024
H = 16   # global heads
HL = 8   # heads per core
HD = 64  # head dim
EL = HL * HD  # 512 local e-dims
N_CORES = 8
TB = 256       # t-block (ctx psum granularity: 2 tc-chunks of 128)
NTB = T // TB  # 8

_CACHED = {}


def legalize_waits(nc, cap=1):
    """Hoist semaphore waits so no instruction carries more than `cap`.

    The cayman 64B ISA instruction format has a single wait slot; this
    container's walrus rejects instructions with more attached waits.
    Tile's sem assignment freely attaches several, so we split the excess
    onto standalone InstEventSemaphore carriers on the same engine.
    Also replaces the tail RANGE_CLEAR with sem-dec updates."""
    import bass_rust

    totals = {}
    names = {}
    for f in nc.m.functions:
        for bb in f.blocks:
            for ins in bb.instructions:
                si = ins.sync_info
                if si is None:
                    continue
                for u in si.on_update or []:
                    if u.sync_type == "semaphore":
                        sign = 1 if u.update_mode in ("sem-inc", "sem-add-imm") else -1
                        totals[u.id] = totals.get(u.id, 0) + sign * u.update_value
                        names[u.id] = u.ant_name

    n = 0
    for f in nc.m.functions:
        for bb in f.blocks:
            insts = bb.instructions
            out = []
            changed = False
            for ins in insts:
                if type(ins).__name__ == "InstISA" and "RANGE_CLEAR" in str(ins):
                    import re

                    m = re.search(r"range_first=(\d+) range_last=(\d+)", str(ins))
                    first, last = int(m.group(1)), int(m.group(2))
                    for sid in range(first, last + 1):
                        tot = totals.get(sid, 0)
                        if tot == 0:
                            continue
                        ev = mybir.InstEventSemaphore(name=f"I-LC{n}", ins=[], outs=[])
                        n += 1
                        ev.engine = ins.engine
                        ev.sync_info = bass_rust.SyncInfo(
                            on_wait=[],
                            on_update=[
                                bass_rust.SyncUpdate(
                                    sync_type="semaphore",
                                    id=sid,
                                    ant_name=names.get(sid, f"sem{sid}"),
                                    update_mode="sem-sub-imm",
                                    update_value=tot,
                                    update_reg=None,
                                )
                            ],
                        )
                        out.append(ev)
                    changed = True
                    continue
                si = ins.sync_info
                ws = list(si.on_wait) if (si is not None and si.on_wait) else []
                if len(ws) > cap:
                    for w in ws[: len(ws) - cap]:
                        ev = mybir.InstEventSemaphore(name=f"I-LW{n}", ins=[], outs=[])
                        n += 1
                        ev.engine = ins.engine
                        ev.sync_info = bass_rust.SyncInfo(on_wait=[w], on_update=[])
                        out.append(ev)
                    si.on_wait = ws[len(ws) - cap :]
                    changed = True
                out.append(ins)
            if changed:
                insts[:] = out
    return n


def build_program(legalize=True, n_tb=NTB, use_dr=True):
    nc = bass.Bass()

    # Host-prepped inputs: x.T and W.T in bf16; biases f32.
    qtd = nc.declare_dram_parameter("qt", [E, T], BF16, isOutput=False)
    ktd = nc.declare_dram_parameter("kt", [E, T], BF16, isOutput=False)
    vtd = nc.declare_dram_parameter("vt", [E, T], BF16, isOutput=False)
    wqtd = nc.declare_dram_parameter("wqt", [E, EL], BF16, isOutput=False)
    wktd = nc.declare_dram_parameter("wkt", [E, EL], BF16, isOutput=False)
    wvtd = nc.declare_dram_parameter("wvt", [E, EL], BF16, isOutput=False)
    wotd = nc.declare_dram_parameter("wot", [EL, E], BF16, isOutput=False)
    bqd = nc.declare_dram_parameter("bq", [EL], F32, isOutput=False)
    bkd = nc.declare_dram_parameter("bk", [EL], F32, isOutput=False)
    outd = nc.declare_dram_parameter("out", [T, E], F32, isOutput=True)

    with tile.TileContext(nc, pool_alloc_mode="queue") as tc:
        with (
            tc.tile_pool(name="singles", bufs=1) as singles,
            tc.tile_pool(name="xt", bufs=2) as xtp,
            tc.tile_pool(name="pt", bufs=8) as ptp,
            tc.tile_pool(name="ctxn", bufs=4) as ctxnp,
            tc.tile_pool(name="ctxT", bufs=2) as ctxTp,
            tc.tile_pool(name="osb", bufs=3) as osbp,
            tc.tile_pool(name="rec", bufs=2) as recp,
            tc.tile_pool(name="sc_ps", bufs=2, space="PSUM") as sc_ps,
            tc.tile_pool(name="ctx_ps", bufs=2, space="PSUM") as ctx_ps,
            tc.tile_pool(name="den_ps", bufs=1, space="PSUM") as den_ps,
            tc.tile_pool(name="misc_ps", bufs=1, space="PSUM") as misc_ps,
        ):
            # ---------------- prologue: weights / biases / consts ----------
            ident = singles.tile([128, 128], BF16)
            make_identity(nc, ident)
            ones = singles.tile([128, 1], BF16)
            nc.vector.memset(ones, 1.0)

            # W.T natural loads: wT[p, a, e'] = WT[a*128 + p, e']
            wqT = singles.tile([128, 8, EL], BF16)
            wkT = singles.tile([128, 8, EL], BF16)
            wvT = singles.tile([128, 8, EL], BF16)
            woT = singles.tile([128, 4, E], BF16)
            bq_sb = singles.tile([128, 4], F32)
            bk_sb = singles.tile([128, 4], F32)

            def load_w_chunked(dst, wd, eng, step=2):
                # paired-e-chunk DMAs so proj matmul e can start early
                for e0 in range(0, 8, step):
                    eng.dma_start(
                        out=dst[:, e0 : e0 + step, :],
                        in_=wd[e0 * 128 : (e0 + step) * 128, :].rearrange(
                            "(a p) e -> p a e", p=128
                        ),
                    )

            nc.sync.dma_start(out=bk_sb, in_=bkd.rearrange("(c p) -> p c", p=128))
            nc.sync.dma_start(out=bq_sb, in_=bqd.rearrange("(c p) -> p c", p=128))

            # qp/kp fp8 DoubleRow tiles, one per 512-t block:
            # [128, kt, c, t]: partition band (h%2)*64 holds head h of chunk
            # c=h//2; kt0 = the 64 head dims, kt1 stays zero.
            qp8 = [singles.tile([128, 2, 4, 512], FP8, name=f"qp8_{i}") for i in range(4)]
            kp8 = [singles.tile([128, 2, 4, 512], FP8, name=f"kp8_{i}") for i in range(4)]

            # vp[s-chunk]: [128 s, 512 e'] bf16
            vp = [singles.tile([128, EL], BF16, name=f"vp_{i}") for i in range(16)]

            def load_xt(xd, blk, tag, eng=None, step=8):
                xt = xtp.tile([128, 8, 512], BF16, tag=tag, name=f"xt_{tag}{blk}")
                sl = xd[:, blk * 512 : (blk + 1) * 512]
                eng = eng or nc.sync
                for e0 in range(0, 8, step):
                    eng.dma_start(
                        out=xt[:, e0 : e0 + step, :],
                        in_=sl[e0 * 128 : (e0 + step) * 128, :].rearrange(
                            "(a p) t -> p a t", p=128
                        ),
                    )
                return xt

            kT = {}
            qT = {}
            # k-side on the HWDGE (SP) queue, q-side on SWDGE (Pool): the two
            # DGE paths generate descriptors in parallel during the prologue.
            kT[0] = load_xt(ktd, 0, "kt", step=2)
            load_w_chunked(wkT, wktd, nc.sync)
            qT[0] = load_xt(qtd, 0, "qt", eng=nc.gpsimd, step=4)
            load_w_chunked(wqT, wqtd, nc.gpsimd, step=4)
            # kt1 zero-fills: blk0 tensors first (gate tb0's first scores)
            for tl in (kp8[0], qp8[0], kp8[1], kp8[2], kp8[3], qp8[1], qp8[2], qp8[3]):
                nc.gpsimd.memset(tl[:, 1, :, :], 0.0)

            def proj_fill_qk(xt, wT, b_sb, dst8, cp, tcols, pool, tag):
                """One 128-e'-chunk x 512-t psum fill + fp8 drain."""
                ps = pool.tile([128, 512], F32, tag=tag)
                for e in range(8):
                    nc.tensor.matmul(
                        ps,
                        lhsT=wT[:, e, cp * 128 : (cp + 1) * 128],
                        rhs=xt[:, e, :],
                        start=(e == 0),
                        stop=(e == 7),
                    )
                nc.vector.tensor_scalar_add(
                    out=dst8[:, 0, cp, :], in0=ps, scalar1=b_sb[:, cp : cp + 1]
                )

            def proj_fill_v(xt, sc):
                """vp[sc] = v-rows sc*128..+128 @ WvT, into misc psum."""
                ps = misc_ps.tile([128, 512], F32, tag="misc")
                for e in range(8):
                    nc.tensor.matmul(
                        ps,
                        lhsT=xt[:, e, (sc % 4) * 128 : (sc % 4 + 1) * 128],
                        rhs=wvT[:, e, :],
                        start=(e == 0),
                        stop=(e == 7),
                    )
                nc.vector.tensor_copy(out=vp[sc], in_=ps)

            # k-proj blk0 + q-proj blk0 up front (scores tb0 need them)
            for cp in range(4):
                proj_fill_qk(kT[0], wkT, bk_sb, kp8[0], cp, 512, ctx_ps, "ctx")
            kT[1] = load_xt(ktd, 1, "kt")
            for cp in range(4):
                proj_fill_qk(qT[0], wqT, bq_sb, qp8[0], cp, 512, ctx_ps, "ctx")

            # remaining loads kick off now; v/wv/wo later consumers
            nc.sync.dma_start(
                out=wvT, in_=wvtd[:, :].rearrange("(a p) e -> p a e", p=128)
            )
            nc.sync.dma_start(
                out=woT, in_=wotd[:, :].rearrange("(a p) e -> p a e", p=128)
            )

            # ---------------- attention + interleaved fillers --------------
            vT = {}

            def dummy_sep(st, opener=False):
                """Tiny bf16 matmul: separates DR groups at different PE row
                tile positions (consecutive DR matmuls with different row
                offsets wedge the device).  Writes an unread den-bank col.
                The per-tb opener (M=128, start=True) opens the den bank's
                zero region — den accumulation then relies on first-touch
                overwrite; later dummies are M=1 accumulates onto col 500."""
                if opener:
                    nc.tensor.matmul(
                        st["den"][:, 500:501],
                        lhsT=ident,
                        rhs=ones,
                        start=True,
                        stop=True,
                        skip_group_check=True,
                    )
                else:
                    nc.tensor.matmul(
                        st["den"][0:1, 500:501],
                        lhsT=ones,
                        rhs=ones,
                        start=False,
                        stop=True,
                        skip_group_check=True,
                    )

            def scores_grp(st, tb, s, g):
                """4 same-band DR score matmuls + exp -> pt tile.
                Band g holds heads 2j+g (j=0..3) at pt cols j*256."""
                dummy_sep(st)
                sc = sc_ps.tile([128, 1024], F32, tag="sc")
                psl = slice(g * 64, g * 64 + 64)
                blk, off = divmod(tb * TB, 512)
                for j in range(4):
                    c = (2 * j + g) // 2  # == j
                    if use_dr:
                        nc.tensor.matmul(
                            sc[:, j * 256 : (j + 1) * 256],
                            lhsT=kp8[s // 4][psl, :, c, (s % 4) * 128 : (s % 4) * 128 + 128],
                            rhs=qp8[blk][psl, :, c, off : off + 256],
                            start=(j % 2 == 0),
                            stop=True,
                            perf_mode=mybir.MatmulPerfMode.DoubleRow,
                            skip_group_check=True,
                        )
                    else:
                        nc.tensor.matmul(
                            sc[:, j * 256 : (j + 1) * 256],
                            lhsT=kp8[s // 4][psl, 0, c, (s % 4) * 128 : (s % 4) * 128 + 128],
                            rhs=qp8[blk][psl, 0, c, off : off + 256],
                            start=(j % 2 == 0),
                            stop=True,
                            skip_group_check=True,
                        )
                pt = ptp.tile([128, 1024], BF16, tag="pt")
                nc.scalar.activation(
                    out=pt, in_=sc, func=mybir.ActivationFunctionType.Exp,
                    scale=0.125,
                )
                return pt

            def pv_half(tb_state, s, g):
                """PV + denom matmuls for band-g heads of (tb, s)."""
                ctx_t, den_t = tb_state["ctx"], tb_state["den"]
                pt = tb_state["pt"][s][g]
                for tc in range(2):
                    for j in range(4):
                        h = 2 * j + g
                        lhsT = pt[:, j * 256 + tc * 128 : j * 256 + tc * 128 + 128]
                        nc.tensor.matmul(
                            ctx_t[tc][:, h * 64 : h * 64 + 64],
                            lhsT=lhsT,
                            rhs=vp[s][:, h * 64 : h * 64 + 64],
                            start=(s == 0 and g == 0 and j == 0),
                            stop=(s == 15),
                            skip_group_check=True,
                        )
                        nc.tensor.matmul(
                            den_t[:, tc * 8 + h : tc * 8 + h + 1],
                            lhsT=lhsT,
                            rhs=ones,
                            start=False,
                            stop=(s == 15),
                            skip_group_check=True,
                        )

            def normalize(tb_state):
                """ctx/den psum -> ctxn bf16 tiles (per-partition recip mult)."""
                rec = recp.tile([128, 16], F32, tag="rec")
                nc.vector.reciprocal(out=rec, in_=tb_state["den"][:, 0:16])
                ctxn = []
                for tc in range(2):
                    cn = ctxnp.tile([128, EL], BF16, tag="ctxn")
                    for h in range(HL):
                        nc.vector.tensor_scalar_mul(
                            out=cn[:, h * 64 : h * 64 + 64],
                            in0=tb_state["ctx"][tc][:, h * 64 : h * 64 + 64],
                            scalar1=rec[:, tc * 8 + h : tc * 8 + h + 1],
                        )
                    ctxn.append(cn)
                tb_state["ctxn"] = ctxn

            def ctxT_fill(tb_state, half):
                """PE-transpose ctxn into ctxT[:, 2 chunks, 256]."""
                if "ctxT" not in tb_state:
                    tb_state["ctxT"] = ctxTp.tile(
                        [128, 4, 256], BF16, tag="ctxT", name=f"ctxT_{tb_state['tb']}"
                    )
                tr = misc_ps.tile([128, 512], BF16, tag="misc")
                for i in range(2):
                    cp = half * 2 + i
                    for tc in range(2):
                        nc.tensor.transpose(
                            tr[:, i * 256 + tc * 128 : i * 256 + tc * 128 + 128],
                            tb_state["ctxn"][tc][:, cp * 128 : (cp + 1) * 128],
                            ident,
                        )
                nc.vector.tensor_copy(
                    out=tb_state["ctxT"][:, half * 2 : half * 2 + 2, :], in_=tr
                )

            def outproj_piece(tb_state, tcc, oh):
                """out[t-chunk, o-half]: 4 matmuls + drain (+DMA when done)."""
                tb = tb_state["tb"]
                ps = misc_ps.tile([128, 512], F32, tag="misc")
                for cp in range(4):
                    nc.tensor.matmul(
                        ps,
                        lhsT=tb_state["ctxT"][:, cp, tcc * 128 : (tcc + 1) * 128],
                        rhs=woT[:, cp, oh * 512 : (oh + 1) * 512],
                        start=(cp == 0),
                        stop=(cp == 3),
                    )
                if oh == 0:
                    tb_state["osb"] = osbp.tile(
                        [128, E], F32, tag="osb", name=f"osb_{tb}_{tcc}"
                    )
                nc.vector.tensor_copy(
                    out=tb_state["osb"][:, oh * 512 : (oh + 1) * 512], in_=ps
                )
                if oh == 1:
                    r0 = tb * TB + tcc * 128
                    nc.sync.dma_start(
                        out=outd[r0 : r0 + 128, :], in_=tb_state["osb"]
                    )

            prev = None  # tb_state of tb-1 (fillers pending)
            for tb in range(n_tb):
                st = {
                    "tb": tb,
                    "ctx": [
                        ctx_ps.tile([128, 512], F32, tag="ctx", name=f"ctx_{tb}_{i}")
                        for i in range(2)
                    ],
                    "den": den_ps.tile([128, 512], F32, tag="den", name=f"den_{tb}"),
                    "pt": {},
                }
                dummy_sep(st, opener=True)
                for s in range(16):
                    ptA = scores_grp(st, tb, s, 0)
                    # --- work block A ---
                    if tb == 0:
                        if s == 1:
                            kT[2] = load_xt(ktd, 2, "kt")
                        if s in (2, 3, 6, 7):  # k blk1 at s2-3, blk2 at s6-7
                            blk = 1 if s < 4 else 2
                            for cp in (0, 2) if s % 2 == 0 else (1, 3):
                                proj_fill_qk(
                                    kT[blk], wkT, bk_sb, kp8[blk], cp, 512,
                                    misc_ps, "misc",
                                )
                        if s in (10, 11):  # k blk3
                            for cp in (0, 2) if s % 2 == 0 else (1, 3):
                                proj_fill_qk(
                                    kT[3], wkT, bk_sb, kp8[3], cp, 512,
                                    misc_ps, "misc",
                                )
                        if s % 4 == 0:
                            vT[s // 4] = load_xt(vtd, s // 4, "vt")
                        if s >= 1:  # v-proj lags one s-chunk
                            proj_fill_v(vT[(s - 1) // 4], s - 1)
                        if s >= 3:
                            pv_half(st, s - 3, 0)
                    else:
                        if s >= 1:
                            pv_half(st, s - 1, 0)
                        if prev is not None:
                            if s == 0:
                                ctxT_fill(prev, 0)
                            elif s == 1:
                                ctxT_fill(prev, 1)
                            elif s in (2, 3, 4, 5):
                                tcc, oh = divmod(s - 2, 2)
                                outproj_piece(prev, tcc, oh)
                    ptB = scores_grp(st, tb, s, 1)
                    st["pt"][s] = (ptA, ptB)
                    # --- work block B ---
                    if tb == 0:
                        if s == 5:
                            kT[3] = load_xt(ktd, 3, "kt")
                        if s >= 3:
                            pv_half(st, s - 3, 1)
                    else:
                        if s >= 1:
                            pv_half(st, s - 1, 1)
                        if tb in (1, 3, 5) and s == 6:
                            blk = (tb + 1) // 2
                            qT[blk] = load_xt(qtd, blk, "qt")
                        if tb in (1, 3, 5) and s in (8, 10, 12, 14):
                            blk = (tb + 1) // 2
                            proj_fill_qk(
                                qT[blk], wqT, bq_sb, qp8[blk], (s - 8) // 2, 512,
                                misc_ps, "misc",
                            )
                if tb == 0:
                    proj_fill_v(vT[3], 15)
                    for s in (13, 14, 15):
                        pv_half(st, s, 0)
                        pv_half(st, s, 1)
                else:
                    pv_half(st, 15, 0)
                    pv_half(st, 15, 1)
                normalize(st)
                st["pt"] = {}  # release references
                prev = st

            # tail: tb7 epilogue
            ctxT_fill(prev, 0)
            ctxT_fill(prev, 1)
            for tcc in range(2):
                for oh in range(2):
                    outproj_piece(prev, tcc, oh)

    if legalize:
        legalize_waits(nc)
    return nc


def _make_in_maps(inputs):
    bf = ml_dtypes.bfloat16
    per_batch = {}
    for b in range(B):
        per_batch[b] = {
            "qt": np.ascontiguousarray(np.asarray(inputs["q"][b]).T).astype(bf),
            "kt": np.ascontiguousarray(np.asarray(inputs["k"][b]).T).astype(bf),
            "vt": np.ascontiguousarray(np.asarray(inputs["v"][b]).T).astype(bf),
        }
    per_half = {}
    for hh in range(2):
        esl = slice(hh * EL, (hh + 1) * EL)
        per_half[hh] = {
            "wqt": np.ascontiguousarray(np.asarray(inputs["Wq"])[esl].T).astype(bf),
            "wkt": np.ascontiguousarray(np.asarray(inputs["Wk"])[esl].T).astype(bf),
            "wvt": np.ascontiguousarray(np.asarray(inputs["Wv"])[esl].T).astype(bf),
            "wot": np.ascontiguousarray(np.asarray(inputs["Wo"])[:, esl].T).astype(bf),
            "bq": np.ascontiguousarray(np.asarray(inputs["bq"])[esl], dtype=np.float32),
            "bk": np.ascontiguousarray(np.asarray(inputs["bk"])[esl], dtype=np.float32),
        }
    in_maps = []
    for c in range(N_CORES):
        b, hh = c // 2, c % 2
        in_maps.append({**per_batch[b], **per_half[hh]})
    return in_maps


def _gather(results, inputs):
    const = (
        np.asarray(inputs["bv"], dtype=np.float32)
        @ np.asarray(inputs["Wo"], dtype=np.float32).T
        + np.asarray(inputs["bo"], dtype=np.float32)
    )
    out = np.empty((B, T, E), dtype=np.float32)
    for b in range(B):
        out[b] = results[2 * b]["out"] + results[2 * b + 1]["out"] + const[None, :]
    return out


def run(inputs, **spmd_kwargs):
    if "nc" not in _CACHED:
        _CACHED["nc"] = build_program()
    nc = _CACHED["nc"]
    in_maps = _make_in_maps(inputs)
    res = run_bass_kernel_spmd(
        nc, in_maps, core_ids=list(range(N_CORES)), **spmd_kwargs
    )
    out = _gather(res.results, inputs)
    return out, res


def kernel(**inputs) -> np.ndarray:
    out, _ = run(inputs)
    return out


# revision 19
# speedup vs baseline: 1.0194x; 1.0016x over previous
"""Multi-head attention (B=4, T=S=2048, E=1024, H=16) on 8 trn2 NeuronCores.

Sharding: core c handles batch b = c // 2 and head-half hh = c % 2
(8 of 16 heads).  The host pre-transposes activations and weights to
bf16 (x.T, W.T) so the kernel needs no on-chip transposes of inputs,
and folds bv/bo into a host-side constant (softmax weights sum to 1,
so the v-bias contributes (bv @ Wo.T) to every row).

On-chip dataflow per core:
  q/k proj -> qp/kp stored as fp8e4 in DoubleRow layout [128, kt2, c4, t]
  (kt1 zeroed), scores.T = kp.T-dot-qp per head via fp8 DoubleRow matmuls
  (cost-model 0.5 cyc/row), exp on ACT from PSUM [128, 1024] 4-head tiles,
  PV in ctx-natural orientation (out [t,64] per head, M=128), softmax
  denominators via ones-column matmuls (N=1), normalize on DVE with
  per-partition reciprocal scalars, PE transpose of ctx, output
  projection to natural [T, E] f32 partials summed on host.
"""

import numpy as np
import ml_dtypes

import concourse.bass as bass
import concourse.mybir as mybir
import concourse.tile as tile
from concourse.bass_utils import run_bass_kernel_spmd
from concourse.masks import make_identity

F32 = mybir.dt.float32
BF16 = mybir.dt.bfloat16
FP8 = mybir.dt.float8e4

B, T, E = 4, 2048, 1024
H = 16   # global heads
HL = 8   # heads per core
HD = 64  # head dim
EL = HL * HD  # 512 local e-dims
N_CORES = 8
TB = 256       # t-block (ctx psum granularity: 2 tc-chunks of 128)
NTB = T // TB  # 8

_CACHED = {}


def legalize_waits(nc, cap=1):
    """Hoist semaphore waits so no instruction carries more than `cap`.

    The cayman 64B ISA instruction format has a single wait slot; this
    container's walrus rejects instructions with more attached waits.
    Tile's sem assignment freely attaches several, so we split the excess
    onto standalone InstEventSemaphore carriers on the same engine.
    Also replaces the tail RANGE_CLEAR with sem-dec updates."""
    import bass_rust

    totals = {}
    names = {}
    for f in nc.m.functions:
        for bb in f.blocks:
            for ins in bb.instructions:
                si = ins.sync_info
                if si is None:
                    continue
                for u in si.on_update or []:
                    if u.sync_type == "semaphore":
                        sign = 1 if u.update_mode in ("sem-inc", "sem-add-imm") else -1
                        totals[u.id] = totals.get(u.id, 0) + sign * u.update_value
                        names[u.id] = u.ant_name

    n = 0
    for f in nc.m.functions:
        for bb in f.blocks:
            insts = bb.instructions
            out = []
            changed = False
            for ins in insts:
                if type(ins).__name__ == "InstISA" and "RANGE_CLEAR" in str(ins):
                    import re

                    m = re.search(r"range_first=(\d+) range_last=(\d+)", str(ins))
                    first, last = int(m.group(1)), int(m.group(2))
                    for sid in range(first, last + 1):
                        tot = totals.get(sid, 0)
                        if tot == 0:
                            continue
                        ev = mybir.InstEventSemaphore(name=f"I-LC{n}", ins=[], outs=[])
                        n += 1
                        ev.engine = ins.engine
                        ev.sync_info = bass_rust.SyncInfo(
                            on_wait=[],
                            on_update=[
                                bass_rust.SyncUpdate(
                                    sync_type="semaphore",
                                    id=sid,
                                    ant_name=names.get(sid, f"sem{sid}"),
                                    update_mode="sem-sub-imm",
                                    update_value=tot,
                                    update_reg=None,
                                )
                            ],
                        )
                        out.append(ev)
                    changed = True
                    continue
                si = ins.sync_info
                ws = list(si.on_wait) if (si is not None and si.on_wait) else []
                if len(ws) > cap:
                    for w in ws[: len(ws) - cap]:
                        ev = mybir.InstEventSemaphore(name=f"I-LW{n}", ins=[], outs=[])
                        n += 1
                        ev.engine = ins.engine
                        ev.sync_info = bass_rust.SyncInfo(on_wait=[w], on_update=[])
                        out.append(ev)
                    si.on_wait = ws[len(ws) - cap :]
                    changed = True
                out.append(ins)
            if changed:
                insts[:] = out
    return n


def build_program(legalize=True, n_tb=NTB, use_dr=True):
    nc = bass.Bass()

    # Host-prepped inputs: x.T and W.T in bf16; biases f32.
    qtd = nc.declare_dram_parameter("qt", [E, T], BF16, isOutput=False)
    ktd = nc.declare_dram_parameter("kt", [E, T], BF16, isOutput=False)
    vtd = nc.declare_dram_parameter("vt", [E, T], BF16, isOutput=False)
    wqtd = nc.declare_dram_parameter("wqt", [E, EL], BF16, isOutput=False)
    wktd = nc.declare_dram_parameter("wkt", [E, EL], BF16, isOutput=False)
    wvtd = nc.declare_dram_parameter("wvt", [E, EL], BF16, isOutput=False)
    wotd = nc.declare_dram_parameter("wot", [EL, E], BF16, isOutput=False)
    bqd = nc.declare_dram_parameter("bq", [EL], F32, isOutput=False)
    bkd = nc.declare_dram_parameter("bk", [EL], F32, isOutput=False)
    outd = nc.declare_dram_parameter("out", [T, E], F32, isOutput=True)

    with tile.TileContext(nc, pool_alloc_mode="queue") as tc:
        with (
            tc.tile_pool(name="singles", bufs=1) as singles,
            tc.tile_pool(name="xt", bufs=2) as xtp,
            tc.tile_pool(name="pt", bufs=8) as ptp,
            tc.tile_pool(name="ctxn", bufs=4) as ctxnp,
            tc.tile_pool(name="ctxT", bufs=2) as ctxTp,
            tc.tile_pool(name="osb", bufs=3) as osbp,
            tc.tile_pool(name="rec", bufs=2) as recp,
            tc.tile_pool(name="sc_ps", bufs=2, space="PSUM") as sc_ps,
            tc.tile_pool(name="ctx_ps", bufs=2, space="PSUM") as ctx_ps,
            tc.tile_pool(name="den_ps", bufs=1, space="PSUM") as den_ps,
            tc.tile_pool(name="misc_ps", bufs=1, space="PSUM") as misc_ps,
        ):
            # ---------------- prologue: weights / biases / consts ----------
            ident = singles.tile([128, 128], BF16)
            make_identity(nc, ident)
            ones = singles.tile([128, 1], BF16)
            nc.vector.memset(ones, 1.0)

            # W.T natural loads: wT[p, a, e'] = WT[a*128 + p, e']
            wqT = singles.tile([128, 8, EL], BF16)
            wkT = singles.tile([128, 8, EL], BF16)
            wvT = singles.tile([128, 8, EL], BF16)
            woT = singles.tile([128, 4, E], BF16)
            bq_sb = singles.tile([128, 4], F32)
            bk_sb = singles.tile([128, 4], F32)

            def load_w_chunked(dst, wd, eng, step=2):
                # paired-e-chunk DMAs so proj matmul e can start early
                for e0 in range(0, 8, step):
                    eng.dma_start(
                        out=dst[:, e0 : e0 + step, :],
                        in_=wd[e0 * 128 : (e0 + step) * 128, :].rearrange(
                            "(a p) e -> p a e", p=128
                        ),
                    )

            nc.sync.dma_start(out=bk_sb, in_=bkd.rearrange("(c p) -> p c", p=128))
            nc.sync.dma_start(out=bq_sb, in_=bqd.rearrange("(c p) -> p c", p=128))

            # qp/kp fp8 DoubleRow tiles, one per 512-t block:
            # [128, kt, c, t]: partition band (h%2)*64 holds head h of chunk
            # c=h//2; kt0 = the 64 head dims, kt1 stays zero.
            qp8 = [singles.tile([128, 2, 4, 512], FP8, name=f"qp8_{i}") for i in range(4)]
            kp8 = [singles.tile([128, 2, 4, 512], FP8, name=f"kp8_{i}") for i in range(4)]

            # vp[s-chunk]: [128 s, 512 e'] bf16
            vp = [singles.tile([128, EL], BF16, name=f"vp_{i}") for i in range(16)]

            def load_xt(xd, blk, tag, eng=None, step=8):
                xt = xtp.tile([128, 8, 512], BF16, tag=tag, name=f"xt_{tag}{blk}")
                sl = xd[:, blk * 512 : (blk + 1) * 512]
                eng = eng or nc.sync
                for e0 in range(0, 8, step):
                    eng.dma_start(
                        out=xt[:, e0 : e0 + step, :],
                        in_=sl[e0 * 128 : (e0 + step) * 128, :].rearrange(
                            "(a p) t -> p a t", p=128
                        ),
                    )
                return xt

            kT = {}
            qT = {}
            # k-side on the HWDGE (SP) queue, q-side on SWDGE (Pool): the two
            # DGE paths generate descriptors in parallel during the prologue.
            kT[0] = load_xt(ktd, 0, "kt", step=2)
            load_w_chunked(wkT, wktd, nc.sync)
            qT[0] = load_xt(qtd, 0, "qt", eng=nc.gpsimd, step=4)
            load_w_chunked(wqT, wqtd, nc.gpsimd, step=4)
            # kt1 zero-fills: blk0 tensors first (gate tb0's first scores)
            for tl in (kp8[0], qp8[0], kp8[1], kp8[2], kp8[3], qp8[1], qp8[2], qp8[3]):
                nc.gpsimd.memset(tl[:, 1, :, :], 0.0)

            def proj_fill_qk(xt, wT, b_sb, dst8, cp, tcols, pool, tag):
                """One 128-e'-chunk x 512-t psum fill + fp8 drain."""
                ps = pool.tile([128, 512], F32, tag=tag)
                for e in range(8):
                    nc.tensor.matmul(
                        ps,
                        lhsT=wT[:, e, cp * 128 : (cp + 1) * 128],
                        rhs=xt[:, e, :],
                        start=(e == 0),
                        stop=(e == 7),
                    )
                nc.vector.tensor_scalar_add(
                    out=dst8[:, 0, cp, :], in0=ps, scalar1=b_sb[:, cp : cp + 1]
                )

            def proj_fill_v(xt, sc):
                """vp[sc] = v-rows sc*128..+128 @ WvT, into misc psum."""
                ps = misc_ps.tile([128, 512], F32, tag="misc")
                for e in range(8):
                    nc.tensor.matmul(
                        ps,
                        lhsT=xt[:, e, (sc % 4) * 128 : (sc % 4 + 1) * 128],
                        rhs=wvT[:, e, :],
                        start=(e == 0),
                        stop=(e == 7),
                    )
                nc.vector.tensor_copy(out=vp[sc], in_=ps)

            # k-proj blk0 + q-proj blk0 up front (scores tb0 need them)
            for cp in range(4):
                proj_fill_qk(kT[0], wkT, bk_sb, kp8[0], cp, 512, ctx_ps, "ctx")
            kT[1] = load_xt(ktd, 1, "kt")
            for cp in range(4):
                proj_fill_qk(qT[0], wqT, bq_sb, qp8[0], cp, 512, ctx_ps, "ctx")

            # remaining loads kick off now; v/wv/wo later consumers
            nc.sync.dma_start(
                out=wvT, in_=wvtd[:, :].rearrange("(a p) e -> p a e", p=128)
            )
            nc.sync.dma_start(
                out=woT, in_=wotd[:, :].rearrange("(a p) e -> p a e", p=128)
            )

            # ---------------- attention + interleaved fillers --------------
            vT = {}

            def dummy_sep(st, opener=False):
                """Tiny bf16 matmul: separates DR groups at different PE row
                tile positions (consecutive DR matmuls with different row
                offsets wedge the device).  Writes an unread den-bank col.
                The per-tb opener (M=128, start=True) opens the den bank's
                zero region — den accumulation then relies on first-touch
                overwrite; later dummies are M=1 accumulates onto col 500."""
                if opener:
                    nc.tensor.matmul(
                        st["den"][:, 500:501],
                        lhsT=ident,
                        rhs=ones,
                        start=True,
                        stop=True,
                        skip_group_check=True,
                    )
                else:
                    nc.tensor.matmul(
                        st["den"][0:1, 500:501],
                        lhsT=ones,
                        rhs=ones,
                        start=False,
                        stop=True,
                        skip_group_check=True,
                    )

            def scores_grp(st, tb, s, g):
                """4 same-band DR score matmuls + exp -> pt tile.
                Band g holds heads 2j+g (j=0..3) at pt cols j*256."""
                dummy_sep(st)
                sc = sc_ps.tile([128, 1024], F32, tag="sc")
                psl = slice(g * 64, g * 64 + 64)
                blk, off = divmod(tb * TB, 512)
                for j in range(4):
                    c = (2 * j + g) // 2  # == j
                    if use_dr:
                        nc.tensor.matmul(
                            sc[:, j * 256 : (j + 1) * 256],
                            lhsT=kp8[s // 4][psl, :, c, (s % 4) * 128 : (s % 4) * 128 + 128],
                            rhs=qp8[blk][psl, :, c, off : off + 256],
                            start=(j % 2 == 0),
                            stop=True,
                            perf_mode=mybir.MatmulPerfMode.DoubleRow,
                            skip_group_check=True,
                        )
                    else:
                        nc.tensor.matmul(
                            sc[:, j * 256 : (j + 1) * 256],
                            lhsT=kp8[s // 4][psl, 0, c, (s % 4) * 128 : (s % 4) * 128 + 128],
                            rhs=qp8[blk][psl, 0, c, off : off + 256],
                            start=(j % 2 == 0),
                            stop=True,
                            skip_group_check=True,
                        )
                pt = ptp.tile([128, 1024], BF16, tag="pt")
                nc.scalar.activation(
                    out=pt, in_=sc, func=mybir.ActivationFunctionType.Exp,
                    scale=0.125,
                )
                return pt

            def pv_half(tb_state, s, g):
                """PV + denom matmuls for band-g heads of (tb, s)."""
                ctx_t, den_t = tb_state["ctx"], tb_state["den"]
                pt = tb_state["pt"][s][g]
                for tc in range(2):
                    for j in range(4):
                        h = 2 * j + g
                        lhsT = pt[:, j * 256 + tc * 128 : j * 256 + tc * 128 + 128]
                        nc.tensor.matmul(
                            ctx_t[tc][:, h * 64 : h * 64 + 64],
                            lhsT=lhsT,
                            rhs=vp[s][:, h * 64 : h * 64 + 64],
                            start=(s == 0 and g == 0 and j == 0),
                            stop=(s == 15),
                            skip_group_check=True,
                        )
                        nc.tensor.matmul(
                            den_t[:, tc * 8 + h : tc * 8 + h + 1],
                            lhsT=lhsT,
                            rhs=ones,
                            start=False,
                            stop=(s == 15),
                            skip_group_check=True,
                        )

            def normalize(tb_state):
                """ctx/den psum -> ctxn bf16 tiles (per-partition recip mult)."""
                rec = recp.tile([128, 16], F32, tag="rec")
                nc.vector.reciprocal(out=rec, in_=tb_state["den"][:, 0:16])
                ctxn = []
                for tc in range(2):
                    cn = ctxnp.tile([128, EL], BF16, tag="ctxn")
                    for h in range(HL):
                        nc.vector.tensor_scalar_mul(
                            out=cn[:, h * 64 : h * 64 + 64],
                            in0=tb_state["ctx"][tc][:, h * 64 : h * 64 + 64],
                            scalar1=rec[:, tc * 8 + h : tc * 8 + h + 1],
                        )
                    ctxn.append(cn)
                tb_state["ctxn"] = ctxn

            def ctxT_fill(tb_state, half):
                """PE-transpose ctxn into ctxT[:, 2 chunks, 256]."""
                if "ctxT" not in tb_state:
                    tb_state["ctxT"] = ctxTp.tile(
                        [128, 4, 256], BF16, tag="ctxT", name=f"ctxT_{tb_state['tb']}"
                    )
                tr = misc_ps.tile([128, 512], BF16, tag="misc")
                for i in range(2):
                    cp = half * 2 + i
                    for tc in range(2):
                        nc.tensor.transpose(
                            tr[:, i * 256 + tc * 128 : i * 256 + tc * 128 + 128],
                            tb_state["ctxn"][tc][:, cp * 128 : (cp + 1) * 128],
                            ident,
                        )
                nc.vector.tensor_copy(
                    out=tb_state["ctxT"][:, half * 2 : half * 2 + 2, :], in_=tr
                )

            def outproj_piece(tb_state, tcc, oh):
                """out[t-chunk, o-half]: 4 matmuls + drain (+DMA when done)."""
                tb = tb_state["tb"]
                ps = misc_ps.tile([128, 512], F32, tag="misc")
                for cp in range(4):
                    nc.tensor.matmul(
                        ps,
                        lhsT=tb_state["ctxT"][:, cp, tcc * 128 : (tcc + 1) * 128],
                        rhs=woT[:, cp, oh * 512 : (oh + 1) * 512],
                        start=(cp == 0),
                        stop=(cp == 3),
                    )
                if oh == 0:
                    tb_state["osb"] = osbp.tile(
                        [128, E], F32, tag="osb", name=f"osb_{tb}_{tcc}"
                    )
                nc.vector.tensor_copy(
                    out=tb_state["osb"][:, oh * 512 : (oh + 1) * 512], in_=ps
                )
                if oh == 1:
                    r0 = tb * TB + tcc * 128
                    nc.sync.dma_start(
                        out=outd[r0 : r0 + 128, :], in_=tb_state["osb"]
                    )

            prev = None  # tb_state of tb-1 (fillers pending)
            for tb in range(n_tb):
                st = {
                    "tb": tb,
                    "ctx": [
                        ctx_ps.tile([128, 512], F32, tag="ctx", name=f"ctx_{tb}_{i}")
                        for i in range(2)
                    ],
                    "den": den_ps.tile([128, 512], F32, tag="den", name=f"den_{tb}"),
                    "pt": {},
                }
                dummy_sep(st, opener=True)
                for s in range(16):
                    ptA = scores_grp(st, tb, s, 0)
                    # --- work block A (between score groups: absorbs the
                    # misc-psum drain latency of the fill below) ---
                    if tb == 0:
                        if s == 0:
                            kT[2] = load_xt(ktd, 2, "kt")
                        if s == 4:
                            kT[3] = load_xt(ktd, 3, "kt")
                        if s < 12:  # k blk 1+s//4, one fill per unit
                            proj_fill_qk(
                                kT[1 + s // 4], wkT, bk_sb, kp8[1 + s // 4],
                                s % 4, 512, misc_ps, "misc",
                            )
                        if s >= 3:
                            pv_half(st, s - 3, 0)
                    else:
                        if s >= 1:
                            pv_half(st, s - 1, 0)
                        if prev is not None:
                            if s == 0:
                                ctxT_fill(prev, 0)
                            elif s == 1:
                                ctxT_fill(prev, 1)
                            elif s in (2, 3, 4, 5):
                                tcc, oh = divmod(s - 2, 2)
                                outproj_piece(prev, tcc, oh)
                    ptB = scores_grp(st, tb, s, 1)
                    st["pt"][s] = (ptA, ptB)
                    # --- work block B ---
                    if tb == 0:
                        if s % 4 == 0:
                            vT[s // 4] = load_xt(vtd, s // 4, "vt")
                        if s >= 1:  # v-proj lags one s-chunk
                            proj_fill_v(vT[(s - 1) // 4], s - 1)
                        if s >= 3:
                            pv_half(st, s - 3, 1)
                    else:
                        if s >= 1:
                            pv_half(st, s - 1, 1)
                        if tb in (1, 3, 5) and s == 6:
                            blk = (tb + 1) // 2
                            qT[blk] = load_xt(qtd, blk, "qt")
                        if tb in (1, 3, 5) and s in (8, 10, 12, 14):
                            blk = (tb + 1) // 2
                            proj_fill_qk(
                                qT[blk], wqT, bq_sb, qp8[blk], (s - 8) // 2, 512,
                                misc_ps, "misc",
                            )
                if tb == 0:
                    proj_fill_v(vT[3], 15)
                    for s in (13, 14, 15):
                        pv_half(st, s, 0)
                        pv_half(st, s, 1)
                else:
                    pv_half(st, 15, 0)
                    pv_half(st, 15, 1)
                normalize(st)
                st["pt"] = {}  # release references
                prev = st

            # tail: tb7 epilogue
            ctxT_fill(prev, 0)
            ctxT_fill(prev, 1)
            for tcc in range(2):
                for oh in range(2):
                    outproj_piece(prev, tcc, oh)

    if legalize:
        legalize_waits(nc)
    return nc


def _make_in_maps(inputs):
    bf = ml_dtypes.bfloat16
    per_batch = {}
    for b in range(B):
        per_batch[b] = {
            "qt": np.ascontiguousarray(np.asarray(inputs["q"][b]).T).astype(bf),
            "kt": np.ascontiguousarray(np.asarray(inputs["k"][b]).T).astype(bf),
            "vt": np.ascontiguousarray(np.asarray(inputs["v"][b]).T).astype(bf),
        }
    per_half = {}
    for hh in range(2):
        esl = slice(hh * EL, (hh + 1) * EL)
        per_half[hh] = {
            "wqt": np.ascontiguousarray(np.asarray(inputs["Wq"])[esl].T).astype(bf),
            "wkt": np.ascontiguousarray(np.asarray(inputs["Wk"])[esl].T).astype(bf),
            "wvt": np.ascontiguousarray(np.asarray(inputs["Wv"])[esl].T).astype(bf),
            "wot": np.ascontiguousarray(np.asarray(inputs["Wo"])[:, esl].T).astype(bf),
            "bq": np.ascontiguousarray(np.asarray(inputs["bq"])[esl], dtype=np.float32),
            "bk": np.ascontiguousarray(np.asarray(inputs["bk"])[esl], dtype=np.float32),
        }
    in_maps = []
    for c in range(N_CORES):
        b, hh = c // 2, c % 2
        in_maps.append({**per_batch[b], **per_half[hh]})
    return in_maps


def _gather(results, inputs):
    const = (
        np.asarray(inputs["bv"], dtype=np.float32)
        @ np.asarray(inputs["Wo"], dtype=np.float32).T
        + np.asarray(inputs["bo"], dtype=np.float32)
    )
    out = np.empty((B, T, E), dtype=np.float32)
    for b in range(B):
        out[b] = results[2 * b]["out"] + results[2 * b + 1]["out"] + const[None, :]
    return out


def run(inputs, **spmd_kwargs):
    if "nc" not in _CACHED:
        _CACHED["nc"] = build_program()
    nc = _CACHED["nc"]
    in_maps = _make_in_maps(inputs)
    res = run_bass_kernel_spmd(
        nc, in_maps, core_ids=list(range(N_CORES)), **spmd_kwargs
    )
    out = _gather(res.results, inputs)
    return out, res


def kernel(**inputs) -> np.ndarray:
    out, _ = run(inputs)
    return out


# revision 21
# speedup vs baseline: 1.0245x; 1.0050x over previous
"""Multi-head attention (B=4, T=S=2048, E=1024, H=16) on 8 trn2 NeuronCores.

Sharding: core c handles batch b = c // 2 and head-half hh = c % 2
(8 of 16 heads).  The host pre-transposes activations and weights to
bf16 (x.T, W.T) so the kernel needs no on-chip transposes of inputs,
and folds bv/bo into a host-side constant (softmax weights sum to 1,
so the v-bias contributes (bv @ Wo.T) to every row).

On-chip dataflow per core:
  q/k proj -> qp/kp stored as fp8e4 in DoubleRow layout [128, kt2, c4, t]
  (kt1 zeroed), scores.T = kp.T-dot-qp per head via fp8 DoubleRow matmuls
  (cost-model 0.5 cyc/row), exp on ACT from PSUM [128, 1024] 4-head tiles,
  PV in ctx-natural orientation (out [t,64] per head, M=128), softmax
  denominators via ones-column matmuls (N=1), normalize on DVE with
  per-partition reciprocal scalars, PE transpose of ctx, output
  projection to natural [T, E] f32 partials summed on host.
"""

import numpy as np
import ml_dtypes

import concourse.bass as bass
import concourse.mybir as mybir
import concourse.tile as tile
from concourse.bass_utils import run_bass_kernel_spmd
from concourse.masks import make_identity

F32 = mybir.dt.float32
BF16 = mybir.dt.bfloat16
FP8 = mybir.dt.float8e4

B, T, E = 4, 2048, 1024
H = 16   # global heads
HL = 8   # heads per core
HD = 64  # head dim
EL = HL * HD  # 512 local e-dims
N_CORES = 8
TB = 256       # t-block (ctx psum granularity: 2 tc-chunks of 128)
NTB = T // TB  # 8

_CACHED = {}


def legalize_waits(nc, cap=1):
    """Hoist semaphore waits so no instruction carries more than `cap`.

    The cayman 64B ISA instruction format has a single wait slot; this
    container's walrus rejects instructions with more attached waits.
    Tile's sem assignment freely attaches several, so we split the excess
    onto standalone InstEventSemaphore carriers on the same engine.
    Also replaces the tail RANGE_CLEAR with sem-dec updates."""
    import bass_rust

    totals = {}
    names = {}
    for f in nc.m.functions:
        for bb in f.blocks:
            for ins in bb.instructions:
                si = ins.sync_info
                if si is None:
                    continue
                for u in si.on_update or []:
                    if u.sync_type == "semaphore":
                        sign = 1 if u.update_mode in ("sem-inc", "sem-add-imm") else -1
                        totals[u.id] = totals.get(u.id, 0) + sign * u.update_value
                        names[u.id] = u.ant_name

    n = 0
    for f in nc.m.functions:
        for bb in f.blocks:
            insts = bb.instructions
            out = []
            changed = False
            for ins in insts:
                if type(ins).__name__ == "InstISA" and "RANGE_CLEAR" in str(ins):
                    import re

                    m = re.search(r"range_first=(\d+) range_last=(\d+)", str(ins))
                    first, last = int(m.group(1)), int(m.group(2))
                    for sid in range(first, last + 1):
                        tot = totals.get(sid, 0)
                        if tot == 0:
                            continue
                        ev = mybir.InstEventSemaphore(name=f"I-LC{n}", ins=[], outs=[])
                        n += 1
                        ev.engine = ins.engine
                        ev.sync_info = bass_rust.SyncInfo(
                            on_wait=[],
                            on_update=[
                                bass_rust.SyncUpdate(
                                    sync_type="semaphore",
                                    id=sid,
                                    ant_name=names.get(sid, f"sem{sid}"),
                                    update_mode="sem-sub-imm",
                                    update_value=tot,
                                    update_reg=None,
                                )
                            ],
                        )
                        out.append(ev)
                    changed = True
                    continue
                si = ins.sync_info
                ws = list(si.on_wait) if (si is not None and si.on_wait) else []
                if len(ws) > cap:
                    for w in ws[: len(ws) - cap]:
                        ev = mybir.InstEventSemaphore(name=f"I-LW{n}", ins=[], outs=[])
                        n += 1
                        ev.engine = ins.engine
                        ev.sync_info = bass_rust.SyncInfo(on_wait=[w], on_update=[])
                        out.append(ev)
                    si.on_wait = ws[len(ws) - cap :]
                    changed = True
                out.append(ins)
            if changed:
                insts[:] = out
    return n


def build_program(legalize=True, n_tb=NTB, use_dr=True):
    nc = bass.Bass()

    # Host-prepped inputs: x.T and W.T in bf16; biases f32.
    qtd = nc.declare_dram_parameter("qt", [E, T], BF16, isOutput=False)
    ktd = nc.declare_dram_parameter("kt", [E, T], BF16, isOutput=False)
    vtd = nc.declare_dram_parameter("vt", [E, T], BF16, isOutput=False)
    wqtd = nc.declare_dram_parameter("wqt", [E, EL], BF16, isOutput=False)
    wktd = nc.declare_dram_parameter("wkt", [E, EL], BF16, isOutput=False)
    wvtd = nc.declare_dram_parameter("wvt", [E, EL], BF16, isOutput=False)
    wotd = nc.declare_dram_parameter("wot", [EL, E], BF16, isOutput=False)
    bqd = nc.declare_dram_parameter("bq", [EL], F32, isOutput=False)
    bkd = nc.declare_dram_parameter("bk", [EL], F32, isOutput=False)
    outd = nc.declare_dram_parameter("out", [T, E], F32, isOutput=True)

    with tile.TileContext(nc, pool_alloc_mode="queue") as tc:
        with (
            tc.tile_pool(name="singles", bufs=1) as singles,
            tc.tile_pool(name="xt", bufs=2) as xtp,
            tc.tile_pool(name="pt", bufs=8) as ptp,
            tc.tile_pool(name="ctxn", bufs=4) as ctxnp,
            tc.tile_pool(name="ctxT", bufs=2) as ctxTp,
            tc.tile_pool(name="osb", bufs=3) as osbp,
            tc.tile_pool(name="rec", bufs=2) as recp,
            tc.tile_pool(name="sc_ps", bufs=2, space="PSUM") as sc_ps,
            tc.tile_pool(name="ctx_ps", bufs=2, space="PSUM") as ctx_ps,
            tc.tile_pool(name="den_ps", bufs=1, space="PSUM") as den_ps,
            tc.tile_pool(name="misc_ps", bufs=1, space="PSUM") as misc_ps,
        ):
            # ---------------- prologue: weights / biases / consts ----------
            ident = singles.tile([128, 128], BF16)
            make_identity(nc, ident)
            ones = singles.tile([128, 1], BF16)
            nc.vector.memset(ones, 1.0)

            # W.T natural loads: wT[p, a, e'] = WT[a*128 + p, e']
            wqT = singles.tile([128, 8, EL], BF16)
            wkT = singles.tile([128, 8, EL], BF16)
            wvT = singles.tile([128, 8, EL], BF16)
            woT = singles.tile([128, 4, E], BF16)
            bq_sb = singles.tile([128, 4], F32)
            bk_sb = singles.tile([128, 4], F32)

            def load_w_chunked(dst, wd, eng, step=2):
                # paired-e-chunk DMAs so proj matmul e can start early
                for e0 in range(0, 8, step):
                    eng.dma_start(
                        out=dst[:, e0 : e0 + step, :],
                        in_=wd[e0 * 128 : (e0 + step) * 128, :].rearrange(
                            "(a p) e -> p a e", p=128
                        ),
                    )

            nc.sync.dma_start(out=bk_sb, in_=bkd.rearrange("(c p) -> p c", p=128))
            nc.sync.dma_start(out=bq_sb, in_=bqd.rearrange("(c p) -> p c", p=128))

            # qp/kp fp8 DoubleRow tiles, one per 512-t block:
            # [128, kt, c, t]: partition band (h%2)*64 holds head h of chunk
            # c=h//2; kt0 = the 64 head dims, kt1 stays zero.
            qp8 = [singles.tile([128, 2, 4, 512], FP8, name=f"qp8_{i}") for i in range(4)]
            kp8 = [singles.tile([128, 2, 4, 512], FP8, name=f"kp8_{i}") for i in range(4)]

            # vp[s-chunk]: [128 s, 512 e'] bf16
            vp = [singles.tile([128, EL], BF16, name=f"vp_{i}") for i in range(16)]

            def load_xt(xd, blk, tag, eng=None, step=8):
                xt = xtp.tile(
                    [128, 8, 512], BF16, tag=tag, name=f"xt_{tag}{blk}",
                    bufs=3 if tag == "kt" else 2,
                )
                sl = xd[:, blk * 512 : (blk + 1) * 512]
                eng = eng or nc.sync
                for e0 in range(0, 8, step):
                    eng.dma_start(
                        out=xt[:, e0 : e0 + step, :],
                        in_=sl[e0 * 128 : (e0 + step) * 128, :].rearrange(
                            "(a p) t -> p a t", p=128
                        ),
                    )
                return xt

            kT = {}
            qT = {}
            # k-side on the HWDGE (SP) queue, q-side on SWDGE (Pool): the two
            # DGE paths generate descriptors in parallel during the prologue.
            kT[0] = load_xt(ktd, 0, "kt", step=2)
            load_w_chunked(wkT, wktd, nc.sync)
            qT[0] = load_xt(qtd, 0, "qt", eng=nc.gpsimd, step=4)
            load_w_chunked(wqT, wqtd, nc.gpsimd, step=4)
            # kt1 zero-fills: blk0 tensors first (gate tb0's first scores)
            for tl in (kp8[0], qp8[0], kp8[1], kp8[2], kp8[3], qp8[1], qp8[2], qp8[3]):
                nc.gpsimd.memset(tl[:, 1, :, :], 0.0)

            def proj_fill_qk(xt, wT, b_sb, dst8, cp, tcols, pool, tag):
                """One 128-e'-chunk x 512-t psum fill + fp8 drain."""
                ps = pool.tile([128, 512], F32, tag=tag)
                for e in range(8):
                    nc.tensor.matmul(
                        ps,
                        lhsT=wT[:, e, cp * 128 : (cp + 1) * 128],
                        rhs=xt[:, e, :],
                        start=(e == 0),
                        stop=(e == 7),
                    )
                nc.vector.tensor_scalar_add(
                    out=dst8[:, 0, cp, :], in0=ps, scalar1=b_sb[:, cp : cp + 1]
                )

            def proj_fill_v(xt, sc):
                """vp[sc] = v-rows sc*128..+128 @ WvT, into misc psum."""
                ps = misc_ps.tile([128, 512], F32, tag="misc")
                for e in range(8):
                    nc.tensor.matmul(
                        ps,
                        lhsT=xt[:, e, (sc % 4) * 128 : (sc % 4 + 1) * 128],
                        rhs=wvT[:, e, :],
                        start=(e == 0),
                        stop=(e == 7),
                    )
                nc.vector.tensor_copy(out=vp[sc], in_=ps)

            # k-proj blk0 + q-proj blk0 up front (scores tb0 need them)
            for cp in range(4):
                proj_fill_qk(kT[0], wkT, bk_sb, kp8[0], cp, 512, ctx_ps, "ctx")
            kT[1] = load_xt(ktd, 1, "kt")
            kT[2] = load_xt(ktd, 2, "kt")
            for cp in range(4):
                proj_fill_qk(qT[0], wqT, bq_sb, qp8[0], cp, 512, ctx_ps, "ctx")

            # remaining loads kick off now; v/wv/wo later consumers
            nc.sync.dma_start(
                out=wvT, in_=wvtd[:, :].rearrange("(a p) e -> p a e", p=128)
            )
            nc.sync.dma_start(
                out=woT, in_=wotd[:, :].rearrange("(a p) e -> p a e", p=128)
            )

            # ---------------- attention + interleaved fillers --------------
            vT = {}

            def dummy_sep(st, opener=False):
                """Tiny bf16 matmul: separates DR groups at different PE row
                tile positions (consecutive DR matmuls with different row
                offsets wedge the device).  Writes an unread den-bank col.
                The per-tb opener (M=128, start=True) opens the den bank's
                zero region — den accumulation then relies on first-touch
                overwrite; later dummies are M=1 accumulates onto col 500."""
                if opener:
                    nc.tensor.matmul(
                        st["den"][:, 500:501],
                        lhsT=ident,
                        rhs=ones,
                        start=True,
                        stop=True,
                        skip_group_check=True,
                    )
                else:
                    nc.tensor.matmul(
                        st["den"][0:1, 500:501],
                        lhsT=ones,
                        rhs=ones,
                        start=False,
                        stop=True,
                        skip_group_check=True,
                    )

            def scores_grp(st, tb, s, g):
                """4 same-band DR score matmuls + exp -> pt tile.
                Band g holds heads 2j+g (j=0..3) at pt cols j*256."""
                dummy_sep(st)
                sc = sc_ps.tile([128, 1024], F32, tag="sc")
                psl = slice(g * 64, g * 64 + 64)
                blk, off = divmod(tb * TB, 512)
                for j in range(4):
                    c = (2 * j + g) // 2  # == j
                    if use_dr:
                        nc.tensor.matmul(
                            sc[:, j * 256 : (j + 1) * 256],
                            lhsT=kp8[s // 4][psl, :, c, (s % 4) * 128 : (s % 4) * 128 + 128],
                            rhs=qp8[blk][psl, :, c, off : off + 256],
                            start=(j % 2 == 0),
                            stop=True,
                            perf_mode=mybir.MatmulPerfMode.DoubleRow,
                            skip_group_check=True,
                        )
                    else:
                        nc.tensor.matmul(
                            sc[:, j * 256 : (j + 1) * 256],
                            lhsT=kp8[s // 4][psl, 0, c, (s % 4) * 128 : (s % 4) * 128 + 128],
                            rhs=qp8[blk][psl, 0, c, off : off + 256],
                            start=(j % 2 == 0),
                            stop=True,
                            skip_group_check=True,
                        )
                pt = ptp.tile([128, 1024], BF16, tag="pt")
                nc.scalar.activation(
                    out=pt, in_=sc, func=mybir.ActivationFunctionType.Exp,
                    scale=0.125,
                )
                return pt

            def pv_half(tb_state, s, g):
                """PV + denom matmuls for band-g heads of (tb, s)."""
                ctx_t, den_t = tb_state["ctx"], tb_state["den"]
                pt = tb_state["pt"][s][g]
                for tc in range(2):
                    for j in range(4):
                        h = 2 * j + g
                        lhsT = pt[:, j * 256 + tc * 128 : j * 256 + tc * 128 + 128]
                        nc.tensor.matmul(
                            ctx_t[tc][:, h * 64 : h * 64 + 64],
                            lhsT=lhsT,
                            rhs=vp[s][:, h * 64 : h * 64 + 64],
                            start=(s == 0 and g == 0 and j == 0),
                            stop=(s == 15),
                            skip_group_check=True,
                        )
                        nc.tensor.matmul(
                            den_t[:, tc * 8 + h : tc * 8 + h + 1],
                            lhsT=lhsT,
                            rhs=ones,
                            start=False,
                            stop=(s == 15),
                            skip_group_check=True,
                        )

            def normalize(tb_state):
                """ctx/den psum -> ctxn bf16 tiles (per-partition recip mult)."""
                rec = recp.tile([128, 16], F32, tag="rec")
                nc.vector.reciprocal(out=rec, in_=tb_state["den"][:, 0:16])
                ctxn = []
                for tc in range(2):
                    cn = ctxnp.tile([128, EL], BF16, tag="ctxn")
                    for h in range(HL):
                        nc.vector.tensor_scalar_mul(
                            out=cn[:, h * 64 : h * 64 + 64],
                            in0=tb_state["ctx"][tc][:, h * 64 : h * 64 + 64],
                            scalar1=rec[:, tc * 8 + h : tc * 8 + h + 1],
                        )
                    ctxn.append(cn)
                tb_state["ctxn"] = ctxn

            def ctxT_fill(tb_state, half):
                """PE-transpose ctxn into ctxT[:, 2 chunks, 256]."""
                if "ctxT" not in tb_state:
                    tb_state["ctxT"] = ctxTp.tile(
                        [128, 4, 256], BF16, tag="ctxT", name=f"ctxT_{tb_state['tb']}"
                    )
                tr = misc_ps.tile([128, 512], BF16, tag="misc")
                for i in range(2):
                    cp = half * 2 + i
                    for tc in range(2):
                        nc.tensor.transpose(
                            tr[:, i * 256 + tc * 128 : i * 256 + tc * 128 + 128],
                            tb_state["ctxn"][tc][:, cp * 128 : (cp + 1) * 128],
                            ident,
                        )
                nc.vector.tensor_copy(
                    out=tb_state["ctxT"][:, half * 2 : half * 2 + 2, :], in_=tr
                )

            def outproj_piece(tb_state, tcc, oh):
                """out[t-chunk, o-half]: 4 matmuls + drain (+DMA when done)."""
                tb = tb_state["tb"]
                ps = misc_ps.tile([128, 512], F32, tag="misc")
                for cp in range(4):
                    nc.tensor.matmul(
                        ps,
                        lhsT=tb_state["ctxT"][:, cp, tcc * 128 : (tcc + 1) * 128],
                        rhs=woT[:, cp, oh * 512 : (oh + 1) * 512],
                        start=(cp == 0),
                        stop=(cp == 3),
                    )
                if oh == 0:
                    tb_state["osb"] = osbp.tile(
                        [128, E], F32, tag="osb", name=f"osb_{tb}_{tcc}"
                    )
                nc.vector.tensor_copy(
                    out=tb_state["osb"][:, oh * 512 : (oh + 1) * 512], in_=ps
                )
                if oh == 1:
                    r0 = tb * TB + tcc * 128
                    nc.sync.dma_start(
                        out=outd[r0 : r0 + 128, :], in_=tb_state["osb"]
                    )

            prev = None  # tb_state of tb-1 (fillers pending)
            for tb in range(n_tb):
                st = {
                    "tb": tb,
                    "ctx": [
                        ctx_ps.tile([128, 512], F32, tag="ctx", name=f"ctx_{tb}_{i}")
                        for i in range(2)
                    ],
                    "den": den_ps.tile([128, 512], F32, tag="den", name=f"den_{tb}"),
                    "pt": {},
                }
                dummy_sep(st, opener=True)
                for s in range(16):
                    ptA = scores_grp(st, tb, s, 0)
                    # --- work block A (between score groups: absorbs the
                    # misc-psum drain latency of the fill below) ---
                    if tb == 0:
                        if s == 0:
                            kT[3] = load_xt(ktd, 3, "kt")
                        if s < 6:  # front-load k fills: blk 1+s//2, 2/unit
                            blk = 1 + s // 2
                            for cp in ((0, 1) if s % 2 == 0 else (2, 3)):
                                proj_fill_qk(
                                    kT[blk], wkT, bk_sb, kp8[blk], cp, 512,
                                    misc_ps, "misc",
                                )
                        if s >= 3:
                            pv_half(st, s - 3, 0)
                    else:
                        if s >= 1:
                            pv_half(st, s - 1, 0)
                        if prev is not None:
                            if s == 0:
                                ctxT_fill(prev, 0)
                            elif s == 1:
                                ctxT_fill(prev, 1)
                            elif s in (2, 3, 4, 5):
                                tcc, oh = divmod(s - 2, 2)
                                outproj_piece(prev, tcc, oh)
                    ptB = scores_grp(st, tb, s, 1)
                    st["pt"][s] = (ptA, ptB)
                    # --- work block B ---
                    if tb == 0:
                        if s % 4 == 0:
                            vT[s // 4] = load_xt(vtd, s // 4, "vt")
                        if s >= 1:  # v-proj lags one s-chunk
                            proj_fill_v(vT[(s - 1) // 4], s - 1)
                        if s >= 3:
                            pv_half(st, s - 3, 1)
                    else:
                        if s >= 1:
                            pv_half(st, s - 1, 1)
                        if tb in (1, 3, 5) and s == 6:
                            blk = (tb + 1) // 2
                            qT[blk] = load_xt(qtd, blk, "qt")
                        if tb in (1, 3, 5) and s in (8, 10, 12, 14):
                            blk = (tb + 1) // 2
                            proj_fill_qk(
                                qT[blk], wqT, bq_sb, qp8[blk], (s - 8) // 2, 512,
                                misc_ps, "misc",
                            )
                if tb == 0:
                    proj_fill_v(vT[3], 15)
                    for s in (13, 14, 15):
                        pv_half(st, s, 0)
                        pv_half(st, s, 1)
                else:
                    pv_half(st, 15, 0)
                    pv_half(st, 15, 1)
                normalize(st)
                st["pt"] = {}  # release references
                prev = st

            # tail: tb7 epilogue
            ctxT_fill(prev, 0)
            ctxT_fill(prev, 1)
            for tcc in range(2):
                for oh in range(2):
                    outproj_piece(prev, tcc, oh)

    if legalize:
        legalize_waits(nc)
    return nc


def _make_in_maps(inputs):
    bf = ml_dtypes.bfloat16
    per_batch = {}
    for b in range(B):
        per_batch[b] = {
            "qt": np.ascontiguousarray(np.asarray(inputs["q"][b]).T).astype(bf),
            "kt": np.ascontiguousarray(np.asarray(inputs["k"][b]).T).astype(bf),
            "vt": np.ascontiguousarray(np.asarray(inputs["v"][b]).T).astype(bf),
        }
    per_half = {}
    for hh in range(2):
        esl = slice(hh * EL, (hh + 1) * EL)
        per_half[hh] = {
            "wqt": np.ascontiguousarray(np.asarray(inputs["Wq"])[esl].T).astype(bf),
            "wkt": np.ascontiguousarray(np.asarray(inputs["Wk"])[esl].T).astype(bf),
            "wvt": np.ascontiguousarray(np.asarray(inputs["Wv"])[esl].T).astype(bf),
            "wot": np.ascontiguousarray(np.asarray(inputs["Wo"])[:, esl].T).astype(bf),
            "bq": np.ascontiguousarray(np.asarray(inputs["bq"])[esl], dtype=np.float32),
            "bk": np.ascontiguousarray(np.asarray(inputs["bk"])[esl], dtype=np.float32),
        }
    in_maps = []
    for c in range(N_CORES):
        b, hh = c // 2, c % 2
        in_maps.append({**per_batch[b], **per_half[hh]})
    return in_maps


def _gather(results, inputs):
    const = (
        np.asarray(inputs["bv"], dtype=np.float32)
        @ np.asarray(inputs["Wo"], dtype=np.float32).T
        + np.asarray(inputs["bo"], dtype=np.float32)
    )
    out = np.empty((B, T, E), dtype=np.float32)
    for b in range(B):
        out[b] = results[2 * b]["out"] + results[2 * b + 1]["out"] + const[None, :]
    return out


def run(inputs, **spmd_kwargs):
    if "nc" not in _CACHED:
        _CACHED["nc"] = build_program()
    nc = _CACHED["nc"]
    in_maps = _make_in_maps(inputs)
    res = run_bass_kernel_spmd(
        nc, in_maps, core_ids=list(range(N_CORES)), **spmd_kwargs
    )
    out = _gather(res.results, inputs)
    return out, res


def kernel(**inputs) -> np.ndarray:
    out, _ = run(inputs)
    return out
